# revision 1
# baseline (speedup 1.0000x reference)
"""MoE routing kernel for Trainium2, 8-core data-parallel.

Problem: nn_MORTM (moe_routing). Full inputs in, full output out.
Sharding: pure data-parallel over tokens (8192 tokens -> 8 cores x 1024).
Each core computes gate softmax + top-2 combine, all 8 routed experts
(dense, weighted by the combine matrix), and the shared expert for its
token slice. No collectives needed; output is a concat of slices.

Matmuls run as float32r (full PE rate at moving dim >= 256) except the
gate matmul, which stays fp32 so top-2 selection matches the fp32
reference ordering.
"""

import numpy as np

import concourse.bacc as bacc
import concourse.bass as bass
import concourse.masks as masks
import concourse.mybir as mybir
import concourse.tile as tile
from concourse.bass_utils import run_bass_kernel_spmd

F32 = mybir.dt.float32
F32R = mybir.dt.float32r
AF = mybir.ActivationFunctionType
ALU = mybir.AluOpType
AX = mybir.AxisListType

N_CORES = 8
USE_SILU = True   # sim_check flips this: CoreSim lacks the Silu LUT
ZERO_BIASES = False  # set by kernel() when every bias input is zero
T = 1024          # tokens per core
D = 1024          # d_model
INTER = 1024      # expert hidden
E = 8             # experts
TB = T // 128     # 128-token blocks
NT = T // 512     # 512-token tiles
DC = D // 128     # d chunks
IC = INTER // 128 # inter chunks
DT = D // 512     # 512-wide d tiles


def r32(ap):
    return ap.bitcast(F32R)


def emit(nc, tc, tensors):
    x_d = tensors["x"]
    gate_d = tensors["gate_w"]
    out_d = tensors["out"]

    xin = x_d.ap().rearrange("(tb p) d -> p tb d", p=128)
    outv = out_d.ap().rearrange("(tb p) d -> p tb d", p=128)

    ctx = tc.nc._emit_ctx  # ExitStack owned by build_nc
    singles = ctx.enter_context(tc.tile_pool(name="singles", bufs=1))
    psum = ctx.enter_context(tc.tile_pool(name="psum", bufs=8, space="PSUM"))
    tmp = ctx.enter_context(tc.tile_pool(name="tmp", bufs=2))
    big = ctx.enter_context(tc.tile_pool(name="big", bufs=1))
    wpool = ctx.enter_context(tc.tile_pool(name="wpool", bufs=24))
    hpool = ctx.enter_context(tc.tile_pool(name="hpool", bufs=1))
    iop = ctx.enter_context(tc.tile_pool(name="iop", bufs=6))

    ident = singles.tile([128, 128], F32)
    masks.make_identity(nc, ident[:])
    onesf = singles.tile([1, 128], F32)
    nc.vector.memset(onesf[:], 1.0)
    ones1 = singles.tile([1, 128], F32R)
    nc.vector.tensor_copy(ones1[:], onesf[:])

    # gate weights transposed: gwT[p, dc, e] = gate_w[e, dc*128+p]
    gwT = singles.tile([128, DC, E], F32)
    for dc in range(DC):
        nc.sync.dma_start(
            gwT[:, dc, :],
            gate_d.ap()[:, dc * 128:(dc + 1) * 128].rearrange("e p -> p e"),
        )

    # routed biases: b1s[p, e, ic] = b1[e, ic*128+p]
    b1s = b3s = sb1s = sb3s = b2r = sb2r = None
    if ZERO_BIASES:
        pass
    else:
        _load_biases = True
    b1s = singles.tile([128, E, IC], F32) if not ZERO_BIASES else None
    b3s = singles.tile([128, E, IC], F32) if not ZERO_BIASES else None
    for e in range(E if not ZERO_BIASES else 0):
        nc.sync.dma_start(
            b1s[:, e, :],
            tensors["b1"].ap()[e].rearrange("(ic p) -> p ic", p=128),
        )
        nc.sync.dma_start(
            b3s[:, e, :],
            tensors["b3"].ap()[e].rearrange("(ic p) -> p ic", p=128),
        )
    if not ZERO_BIASES:
        sb1s = singles.tile([128, IC], F32)
        nc.sync.dma_start(
            sb1s[:], tensors["sb1"].ap().rearrange("(ic p) -> p ic", p=128)
        )
        sb3s = singles.tile([128, IC], F32)
        nc.sync.dma_start(
            sb3s[:], tensors["sb3"].ap().rearrange("(ic p) -> p ic", p=128)
        )
    # row biases for the second matmul (added via K=1 matmul broadcast);
    # expert j's row lives on partition j.
    if not ZERO_BIASES:
        b2r = singles.tile([E, D], F32R)
        nc.sync.dma_start(b2r[:], tensors["b2"].ap().bitcast(F32R))
        sb2r = singles.tile([1, D], F32R)
        nc.sync.dma_start(
            sb2r[:],
            tensors["sb2"].ap().rearrange("(o d) -> o d", o=1).bitcast(F32R),
        )

    xt = big.tile([128, DC, T], F32R)     # xt[p, dc, t] = x[t, dc*128+p]
    comb = big.tile([128, TB, E], F32)   # combine matrix
    comb_t = (
        None if ZERO_BIASES else big.tile([8, T], F32R)
    )  # combine transposed [expert, token]

    # ---- per token block: load x, PE-transpose (fp32 stage + f32r copy),
    #      gate scores from the fp32 stage -> softmax -> top2 -> combine ----
    xpool_cm = tc.tile_pool(name="xnat", bufs=2)
    xpool = xpool_cm.__enter__()
    for tb in range(TB):
        xnat = xpool.tile([128, D], F32, tag="xnat")
        nc.sync.dma_start(xnat[:], xin[:, tb, :])
        xstage = xpool.tile([128, DC, 128], F32, tag="xstage")
        for dc in range(DC):
            pt = psum.tile([128, 512], F32, tag="ps")
            nc.tensor.transpose(
                pt[:, :128], xnat[:, dc * 128:(dc + 1) * 128], ident[:]
            )
            nc.vector.tensor_copy(xstage[:, dc, :], pt[:, :128])
            nc.vector.tensor_copy(xt[:, dc, tb * 128:(tb + 1) * 128], xstage[:, dc, :])
        ps = psum.tile([128, 512], F32, tag="ps")
        for dc in range(DC):
            nc.tensor.matmul(
                ps[:, :E],
                xstage[:, dc, :],
                gwT[:, dc, :],
                start=(dc == 0),
                stop=(dc == DC - 1),
            )
        nmx = tmp.tile([128, 1], F32, tag="nmx")
        nc.vector.tensor_reduce(nmx[:], ps[:, :E], axis=AX.X, op=ALU.max, negate=True)
        ex = tmp.tile([128, E], F32, tag="ex")
        nc.scalar.activation(ex[:], ps[:, :E], AF.Exp, bias=nmx[:])
        ssum = tmp.tile([128, 1], F32, tag="ssum")
        nc.vector.tensor_reduce(ssum[:], ex[:], axis=AX.X, op=ALU.add)
        rs = tmp.tile([128, 1], F32, tag="rs")
        nc.vector.reciprocal(rs[:], ssum[:])
        probs = tmp.tile([128, E], F32, tag="probs")
        nc.vector.tensor_scalar_mul(probs[:], ex[:], rs[:])
        m8 = tmp.tile([128, 8], F32, tag="m8")
        nc.vector.max(m8[:], probs[:])
        msk = tmp.tile([128, E], F32, tag="msk")
        nc.vector.tensor_scalar(msk[:], probs[:], m8[:, 1:2], None, op0=ALU.is_ge)
        nc.vector.tensor_mul(comb[:, tb, :], probs[:], msk[:])
        if not ZERO_BIASES:
            ptc = psum.tile([128, 512], F32, tag="ps")
            nc.tensor.transpose(ptc[:8, :128], comb[:, tb, :], ident[:])
            nc.vector.tensor_copy(
                comb_t[:, tb * 128:(tb + 1) * 128], ptc[:8, :128]
            )

    xpool_cm.__exit__(None, None, None)

    # ---- experts: shared first (j == -1), then routed 0..7 ----
    for j in range(-1, E):
        shared = j < 0
        # double-buffered so expert j+1's h-phase overlaps expert j's y-phase
        hbuf = hpool.tile([128, IC, T], F32R, tag="hbuf")
        if shared:
            w1d, w3d, w2d = tensors["sw1"].ap(), tensors["sw3"].ap(), tensors["sw2"].ap()
        else:
            w1d, w3d, w2d = (
                tensors["w1"].ap()[j],
                tensors["w3"].ap()[j],
                tensors["w2"].ap()[j],
            )

        s1 = []
        s3 = []
        for dc in range(DC):
            t1 = wpool.tile([128, INTER], F32R, tag="wslab")
            nc.sync.dma_start(t1[:], w1d[dc * 128:(dc + 1) * 128, :].bitcast(F32R))
            s1.append(t1)
            t3 = wpool.tile([128, INTER], F32R, tag="wslab")
            nc.sync.dma_start(t3[:], w3d[dc * 128:(dc + 1) * 128, :].bitcast(F32R))
            s3.append(t3)

        # h = silu(x @ w1 + b1) * (x @ w3 + b3), transposed layout [inter, tok]
        for nt in range(NT):
            tsl = slice(nt * 512, (nt + 1) * 512)
            for icp in range(IC // 2):
                phs = []
                for k in range(2):
                    ic = icp * 2 + k
                    icb = slice(ic * 128, (ic + 1) * 128)
                    p1 = psum.tile([128, 512], F32, tag="ps")
                    p3 = psum.tile([128, 512], F32, tag="ps")
                    for dc in range(DC):
                        st, sp = dc == 0, dc == DC - 1
                        nc.tensor.matmul(
                            p1[:], s1[dc][:, icb], xt[:, dc, tsl],
                            start=st, stop=sp,
                        )
                        nc.tensor.matmul(
                            p3[:], s3[dc][:, icb], xt[:, dc, tsl],
                            start=st, stop=sp,
                        )
                    phs.append((ic, p1, p3))
                for ic, p1, p3 in phs:
                    hs = tmp.tile([128, 512], F32, tag="hs")
                    if ZERO_BIASES:
                        if USE_SILU:
                            nc.scalar.activation(hs[:], p1[:], AF.Silu)
                        else:
                            sg = tmp.tile([128, 512], F32, tag="sg")
                            nc.scalar.activation(sg[:], p1[:], AF.Sigmoid)
                            nc.vector.tensor_mul(hs[:], sg[:], p1[:])
                        nc.vector.tensor_mul(hbuf[:, ic, tsl], hs[:], p3[:])
                        continue
                    b1c = sb1s[:, ic:ic + 1] if shared else b1s[:, j, ic:ic + 1]
                    b3c = sb3s[:, ic:ic + 1] if shared else b3s[:, j, ic:ic + 1]
                    t3v = tmp.tile([128, 512], F32, tag="t3v")
                    nc.vector.tensor_scalar_add(t3v[:], p3[:], b3c)
                    if USE_SILU:
                        nc.scalar.activation(hs[:], p1[:], AF.Silu, bias=b1c)
                    else:  # CoreSim has no Silu: silu(v) = v * sigmoid(v)
                        sg = tmp.tile([128, 512], F32, tag="sg")
                        nc.scalar.activation(sg[:], p1[:], AF.Sigmoid, bias=b1c)
                        t1v = tmp.tile([128, 512], F32, tag="t1v")
                        nc.vector.tensor_scalar_add(t1v[:], p1[:], b1c)
                        nc.vector.tensor_mul(hs[:], sg[:], t1v[:])
                    nc.vector.tensor_mul(hbuf[:, ic, tsl], hs[:], t3v[:])

        # second matmul back to natural layout + bias + weighted accumulate
        s2 = []
        for ic in range(IC):
            t2 = wpool.tile([128, D], F32R, tag="wslab")
            nc.sync.dma_start(t2[:], w2d[ic * 128:(ic + 1) * 128, :].bitcast(F32R))
            s2.append(t2)
        b2row = None if ZERO_BIASES else (sb2r[0:1, :] if shared else b2r[j:j + 1, :])
        for tb in range(TB):
            tbb = slice(tb * 128, (tb + 1) * 128)
            for dt in range(DT):
                dsl = slice(dt * 512, (dt + 1) * 512)
                py = psum.tile([128, 512], F32, tag="ps")
                for ic in range(IC):
                    nc.tensor.matmul(
                        py[:], hbuf[:, ic, tbb], s2[ic][:, dsl],
                        start=(ic == 0),
                        stop=(ic == IC - 1) and (ZERO_BIASES or not shared),
                    )
                if not ZERO_BIASES and shared:
                    # shared bias + sum_j combine[t,j]*b2[j,:] (K=8 matmul)
                    nc.tensor.matmul(
                        py[:], ones1[:], b2row[:, dsl],
                        start=False, stop=False,
                    )
                    nc.tensor.matmul(
                        py[:], comb_t[:, tbb], b2r[:, dsl],
                        start=False, stop=True,
                    )
                st = iop.tile([128, 512], F32, tag="st")
                if shared:
                    nc.scalar.copy(st[:], py[:])
                else:
                    # out slice += combine[:, j] * py  (RMW through DRAM)
                    nc.vector.tensor_scalar_mul(st[:], py[:], comb[:, tb, j:j + 1])
                    rd = iop.tile([128, 512], F32, tag="rd")
                    nc.sync.dma_start(rd[:], outv[:, tb, dsl])
                    nc.vector.tensor_tensor(st[:], st[:], rd[:], op=ALU.add)
                nc.sync.dma_start(outv[:, tb, dsl], st[:])


def declare(nc):
    tensors = {
        "x": nc.dram_tensor("x", [T, D], F32, kind="ExternalInput"),
        "gate_w": nc.dram_tensor("gate_w", [E, D], F32, kind="ExternalInput"),
        "w1": nc.dram_tensor("w1", [E, D, INTER], F32, kind="ExternalInput"),
        "b1": nc.dram_tensor("b1", [E, INTER], F32, kind="ExternalInput"),
        "w2": nc.dram_tensor("w2", [E, INTER, D], F32, kind="ExternalInput"),
        "b2": nc.dram_tensor("b2", [E, D], F32, kind="ExternalInput"),
        "w3": nc.dram_tensor("w3", [E, D, INTER], F32, kind="ExternalInput"),
        "b3": nc.dram_tensor("b3", [E, INTER], F32, kind="ExternalInput"),
        "sw1": nc.dram_tensor("sw1", [D, INTER], F32, kind="ExternalInput"),
        "sb1": nc.dram_tensor("sb1", [INTER], F32, kind="ExternalInput"),
        "sw2": nc.dram_tensor("sw2", [INTER, D], F32, kind="ExternalInput"),
        "sb2": nc.dram_tensor("sb2", [D], F32, kind="ExternalInput"),
        "sw3": nc.dram_tensor("sw3", [D, INTER], F32, kind="ExternalInput"),
        "sb3": nc.dram_tensor("sb3", [INTER], F32, kind="ExternalInput"),
        "out": nc.dram_tensor("out", [T, D], F32, kind="ExternalOutput"),
    }
    return tensors


def build_nc(num_devices=N_CORES):
    from contextlib import ExitStack

    nc = bacc.Bacc(
        "TRN2", target_bir_lowering=False, debug=False, num_devices=num_devices
    )
    tensors = declare(nc)
    with tile.TileContext(nc) as tc:
        with ExitStack() as es:
            nc._emit_ctx = es
            emit(nc, tc, tensors)
    nc.compile()
    return nc


def make_in_maps(inputs):
    x = np.ascontiguousarray(
        np.asarray(inputs["x"], dtype=np.float32).reshape(-1, D)
    )
    shared_names = [
        "gate_w", "w1", "b1", "w2", "b2", "w3", "b3",
        "sw1", "sb1", "sw2", "sb2", "sw3", "sb3",
    ]
    shared = {
        k: np.ascontiguousarray(np.asarray(inputs[k], dtype=np.float32))
        for k in shared_names
    }
    in_maps = []
    for c in range(N_CORES):
        m = dict(shared)
        m["x"] = np.ascontiguousarray(x[c * T:(c + 1) * T])
        in_maps.append(m)
    return in_maps


def kernel(**inputs) -> np.ndarray:
    global ZERO_BIASES
    ZERO_BIASES = all(
        not np.any(np.asarray(inputs[k]))
        for k in ("b1", "b2", "b3", "sb1", "sb2", "sb3")
    )
    nc = build_nc()
    in_maps = make_in_maps(inputs)
    res = run_bass_kernel_spmd(nc, in_maps, core_ids=list(range(N_CORES)))
    out = np.concatenate([res.results[c]["out"] for c in range(N_CORES)], axis=0)
    return out.reshape(np.asarray(inputs["x"]).shape)



# revision 18
# speedup vs baseline: 1.6815x; 1.6815x over previous
"""MoE top-2 routing kernel for Trainium2, 8-core data-parallel, sparse.

Problem: nn_MORTM (moe_routing). Full inputs in, full output out.

Sharding: data-parallel over tokens (8192 tokens -> 8 cores x 1024).
Each core:
  1. fp32 gate matmul (host-pretransposed xT_f32 streamed as moving
     operand against stationary xT tiles) -> softmax -> top-2 mask.
  2. Builds per-expert token-index and combine-weight lists on-device
     (PE transpose to expert-major rows, gpsimd sparse_gather compaction).
  3. For each expert, dma_gather (transposed, bf16) pulls only that
     expert's routed tokens (static capacity CAP=384 >= observed max
     count ~282), runs the SwiGLU in bf16 (weights moving for mm1/mm3 so
     hT comes out expert-contraction-major; hT stationary for mm2 so y
     comes out token-major), scales rows by the gathered combine weight,
     and dma_scatter_adds the result into the output in DRAM.
  4. The shared expert runs densely on the core's 1024 tokens in bf16
     and writes the output first (scatter-adds then accumulate on top).

Pad slots in each expert's list have their gather index clamped to 0 and
combine weight forced to 0, so they contribute +0.0 to token 0.

Fallback: the original dense kernel (all 8 experts weighted by the
combine matrix) is kept for non-zero biases or capacity overflow.
"""

import numpy as np

import concourse.bacc as bacc
import concourse.bass as bass
import concourse.masks as masks
import concourse.mybir as mybir
import concourse.tile as tile
from concourse.bass_utils import run_bass_kernel_spmd

F32 = mybir.dt.float32
F32R = mybir.dt.float32r
BF16 = mybir.dt.bfloat16
I16 = mybir.dt.int16
U32 = mybir.dt.uint32
AF = mybir.ActivationFunctionType
ALU = mybir.AluOpType
AX = mybir.AxisListType

N_CORES = 8
T = 1024          # tokens per core
D = 1024          # d_model
INTER = 1024      # expert hidden
E = 8             # experts
TB = T // 128     # 128-token blocks
DC = D // 128     # d chunks
IC = INTER // 128 # inter chunks
CAP = 384         # per-(core, expert) routed-token capacity (3 pair tiles)
PT = CAP // 128   # pair tiles per expert
NW = CAP // 16    # wrapped index columns

ZERO_BIASES = False  # kept for test.py compat; set by kernel()
DEBUG_DUMPS = False  # sim_check: dump intermediates to DRAM scratch
DEBUG_KIND = "Internal"  # "ExternalOutput" to fetch dumps from HW
USE_SILU = True   # sim_check flips this: CoreSim lacks the Silu LUT


# ---------------------------------------------------------------- sparse path


def emit_sparse(nc, tc, tn):
    ctx = tc.nc._emit_ctx
    singles = ctx.enter_context(tc.tile_pool(name="singles", bufs=1))
    psum = ctx.enter_context(tc.tile_pool(name="psum", bufs=8, space="PSUM"))
    tmp = ctx.enter_context(tc.tile_pool(name="tmp", bufs=4))
    wpool = ctx.enter_context(tc.tile_pool(name="wpool", bufs=28 if USE_SILU else 22))
    idxp = ctx.enter_context(tc.tile_pool(name="idxp", bufs=1))
    xgp = ctx.enter_context(tc.tile_pool(name="xgp", bufs=2))
    hsp = ctx.enter_context(tc.tile_pool(name="hsp", bufs=2))
    yfp = ctx.enter_context(tc.tile_pool(name="yfp", bufs=2))
    iop = ctx.enter_context(tc.tile_pool(name="iop", bufs=2))
    shp = ctx.enter_context(tc.tile_pool(name="shp", bufs=1))

    ident = singles.tile([128, 128], F32)
    masks.make_identity(nc, ident[:])

    # gate weights transposed: gwT[p, dc, e] = gate_w[e, dc*128+p]
    gwT = singles.tile([128, DC, E], F32)
    for dc in range(DC):
        nc.sync.dma_start(
            gwT[:, dc, :],
            tn["gate_w"].ap()[:, dc * 128:(dc + 1) * 128].rearrange("e p -> p e"),
        )
    # tokv[p, tb] = tb*128 + p + 1  (host constant)
    tokv = singles.tile([128, TB], F32)
    nc.sync.dma_start(tokv[:], tn["tokv"].ap().rearrange("(tb p) -> p tb", p=128))

    # expert-major value rows for sparse_gather (built during gate phase)
    vT = singles.tile([E, T], F32)    # token id + 1 rows (-1 = unrouted)
    cvT = singles.tile([E, T], F32)   # combine weight + 1 rows (-1 = unrouted)

    # ---- gate phase: scores -> softmax -> top2 -> v/cv rows ----
    with tc.tile_pool(name="gatep", bufs=1) as gatep:
        xtf = gatep.tile([128, DC, T], F32)
        nc.sync.dma_start(xtf[:], tn["xT_f32"].ap().rearrange("(dc p) t -> p dc t", p=128))
        for tb in range(TB):
            tsl = slice(tb * 128, (tb + 1) * 128)
            ps = psum.tile([128, 512], F32, tag="ps")
            for dc in range(DC):
                nc.tensor.matmul(
                    ps[:, :E],
                    xtf[:, dc, tsl],
                    gwT[:, dc, :],
                    start=(dc == 0),
                    stop=(dc == DC - 1),
                )
            nmx = tmp.tile([128, 1], F32, tag="nmx")
            nc.vector.tensor_reduce(nmx[:], ps[:, :E], axis=AX.X, op=ALU.max, negate=True)
            ex = tmp.tile([128, E], F32, tag="ex")
            nc.scalar.activation(ex[:], ps[:, :E], AF.Exp, bias=nmx[:])
            ssum = tmp.tile([128, 1], F32, tag="ssum")
            nc.vector.tensor_reduce(ssum[:], ex[:], axis=AX.X, op=ALU.add)
            rs = tmp.tile([128, 1], F32, tag="rs")
            nc.vector.reciprocal(rs[:], ssum[:])
            probs = tmp.tile([128, E], F32, tag="probs")
            nc.vector.tensor_scalar_mul(probs[:], ex[:], rs[:])
            m8 = tmp.tile([128, 8], F32, tag="m8")
            nc.vector.max(m8[:], probs[:])
            msk = tmp.tile([128, E], F32, tag="msk")
            nc.vector.tensor_scalar(msk[:], probs[:], m8[:, 1:2], None, op0=ALU.is_ge)
            # v = msk * (tok + 1) - 1 ; cv = msk * (probs + 1) - 1
            v = tmp.tile([128, E], F32, tag="v")
            nc.vector.tensor_scalar(v[:], msk[:], tokv[:, tb:tb + 1], -1.0,
                                    op0=ALU.mult, op1=ALU.add)
            p1 = tmp.tile([128, E], F32, tag="p1")
            nc.vector.tensor_scalar_add(p1[:], probs[:], 1.0)
            cv = tmp.tile([128, E], F32, tag="cv")
            nc.vector.tensor_tensor(cv[:], msk[:], p1[:], op=ALU.mult)
            nc.vector.tensor_scalar_add(cv[:], cv[:], -1.0)
            # transpose to expert-major rows
            ptv = psum.tile([128, 512], F32, tag="ps")
            nc.tensor.transpose(ptv[:E, :128], v[:], ident[:])
            nc.vector.tensor_copy(vT[:, tsl], ptv[:E, :128])
            ptc = psum.tile([128, 512], F32, tag="ps")
            nc.tensor.transpose(ptc[:E, :128], cv[:], ident[:])
            nc.vector.tensor_copy(cvT[:, tsl], ptc[:E, :128])

        # per-expert routed counts (while the gate pool is still open)
        mask8 = gatep.tile([E, T], F32)
        nc.vector.tensor_scalar(mask8[:], cvT[:], 0.0, None, op0=ALU.is_ge)
        n8 = gatep.tile([E, 1], F32)
        nc.vector.tensor_reduce(n8[:], mask8[:], axis=AX.X, op=ALU.add)
        nf_d = nc.dram_tensor("nf_scratch", [16, E], F32, kind="Internal")
        for r in range(16):
            nc.sync.dma_start(nf_d.ap()[r], n8[:, 0:1])

    # ---- index build for all experts (gpsimd, overlaps shared expert) ----
    # SBUF [1, T] rows can't be re-partitioned directly (illegal partition
    # step); bounce the expert-major rows through DRAM scratch.
    vT_d = nc.dram_tensor("vT_scratch", [E, T], F32, kind=DEBUG_KIND)
    cvT_d = nc.dram_tensor("cvT_scratch", [E, T], F32, kind=DEBUG_KIND)
    nc.sync.dma_start(vT_d.ap(), vT[:])
    nc.sync.dma_start(cvT_d.ap(), cvT[:])
    v16 = idxp.tile([16, E, T // 16], F32)
    c16 = idxp.tile([16, E, T // 16], F32)
    idxf = idxp.tile([16, E, NW], F32)    # compacted token ids (f32)
    cf = idxp.tile([16, E, NW], F32)      # compacted combine+1 (f32)
    nfound = idxp.tile([1, 2 * E], U32)
    idxsc = idxp.tile([128, E, NW], I16)  # scatter idx (clamped >= 0)
    cadj = idxp.tile([128, E, PT], F32)   # combine weights per pair tile

    # validm = (logical slot < n) masks off the sparse_gather tail,
    # which is junk on HW ucode (the interpreter writes -1 there,
    # hardware does not).
    iotaw = idxp.tile([16, NW], F32)
    nc.sync.dma_start(iotaw[:], tn["iotaw"].ap())

    for e in range(E):
        # relayout row -> [16, 64], dest[r, f] = vT[e, 64r + f]
        nc.sync.dma_start(
            v16[:, e, :],
            vT_d.ap()[e].rearrange("(r f) -> r f", r=16),
        )
        nc.sync.dma_start(
            c16[:, e, :],
            cvT_d.ap()[e].rearrange("(r f) -> r f", r=16),
        )
        nc.vector.memset(idxf[:, e, :], -1.0)
        nc.vector.memset(cf[:, e, :], -1.0)
        nc.gpsimd.sparse_gather(
            idxf[:, e, :], v16[:, e, :], num_found=nfound[:, 2 * e:2 * e + 1]
        )
        nc.gpsimd.sparse_gather(
            cf[:, e, :], c16[:, e, :], num_found=nfound[:, 2 * e + 1:2 * e + 2]
        )
        nfb = tmp.tile([16, 1], F32, tag="nfb")
        nc.sync.dma_start(nfb[:], nf_d.ap()[:, e:e + 1])
        validm = tmp.tile([16, NW], I16, tag="validm")
        nc.vector.tensor_scalar(validm[:], iotaw[:], nfb[:, 0:1], None,
                                op0=ALU.is_lt)
        # select valid lanes only (junk-tail/NaN proof); pads become
        # token 0 with weight 0
        idxsafe = tmp.tile([16, NW], F32, tag="idxsafe")
        nc.vector.memset(idxsafe[:], 0.0)
        nc.vector.copy_predicated(idxsafe[:], validm[:], idxf[:, e, :])
        cfsafe = tmp.tile([16, NW], F32, tag="cfsafe")
        nc.vector.memset(cfsafe[:], 0.0)
        nc.vector.copy_predicated(cfsafe[:], validm[:], cf[:, e, :])
        i16t = tmp.tile([16, NW], I16, tag="i16t")
        nc.vector.tensor_copy(i16t[:], idxsafe[:])
        for g in range(8):
            nc.sync.dma_start(idxsc[16 * g:16 * (g + 1), e, :], i16t[:])
        # c_nat[p, t] = cfsafe[p%16, 8t + p//16]
        cfv = cfsafe[:].rearrange("r (t q) -> r q t", q=8)
        for q in range(8):
            nc.sync.dma_start(cadj[16 * q:16 * (q + 1), e, :], cfv[:, q, :])

    if DEBUG_DUMPS:
        idx_dbg = nc.dram_tensor("idx_dbg", [128, E, NW], I16, kind=DEBUG_KIND)
        nc.sync.dma_start(idx_dbg.ap(), idxsc[:])
        cadj_dbg = nc.dram_tensor("cadj_dbg", [128, E, PT], F32, kind=DEBUG_KIND)
        nc.sync.dma_start(cadj_dbg.ap(), cadj[:])
        cf_dbg = nc.dram_tensor("cf_dbg", [16, E, NW], F32, kind=DEBUG_KIND)
        nc.sync.dma_start(cf_dbg.ap(), cf[:])
        idxf_dbg = nc.dram_tensor("idxf_dbg", [16, E, NW], F32, kind=DEBUG_KIND)
        nc.sync.dma_start(idxf_dbg.ap(), idxf[:])
        c16_dbg = nc.dram_tensor("c16_dbg", [16, E, T // 16], F32, kind=DEBUG_KIND)
        nc.sync.dma_start(c16_dbg.ap(), c16[:])

    # ---- shared expert (dense, bf16), writes out first ----
    xts = shp.tile([128, DC, T], BF16)
    nc.sync.dma_start(xts[:], tn["xT_bf"].ap().rearrange("(dc p) t -> p dc t", p=128))
    ss1 = []
    ss3 = []
    for dc in range(DC):
        t1 = wpool.tile([128, INTER], BF16, tag="wslab")
        nc.sync.dma_start(t1[:], tn["sw1"].ap()[dc * 128:(dc + 1) * 128, :])
        ss1.append(t1)
        t3 = wpool.tile([128, INTER], BF16, tag="wslab")
        nc.sync.dma_start(t3[:], tn["sw3"].ap()[dc * 128:(dc + 1) * 128, :])
        ss3.append(t3)
    hsh = shp.tile([128, IC, T], BF16)
    for ic in range(IC):
        icb = slice(ic * 128, (ic + 1) * 128)
        for nt in range(2):
            tsl = slice(nt * 512, (nt + 1) * 512)
            p1 = psum.tile([128, 512], F32, tag="ps")
            p3 = psum.tile([128, 512], F32, tag="ps")
            for dc in range(DC):
                st, sp = dc == 0, dc == DC - 1
                nc.tensor.matmul(p1[:], ss1[dc][:, icb], xts[:, dc, tsl], start=st, stop=sp)
                nc.tensor.matmul(p3[:], ss3[dc][:, icb], xts[:, dc, tsl], start=st, stop=sp)
            sg = tmp.tile([128, 512], F32, tag="sg")
            if USE_SILU:
                nc.scalar.activation(sg[:], p1[:], AF.Silu)
            else:
                sgm = tmp.tile([128, 512], F32, tag="sgm")
                nc.scalar.activation(sgm[:], p1[:], AF.Sigmoid)
                nc.vector.tensor_tensor(sg[:], sgm[:], p1[:], op=ALU.mult)
            nc.vector.tensor_tensor(hsh[:, ic, tsl], sg[:], p3[:], op=ALU.mult)
    ss2 = []
    for ic in range(IC):
        t2 = wpool.tile([128, D], BF16, tag="wslab")
        nc.sync.dma_start(t2[:], tn["sw2"].ap()[ic * 128:(ic + 1) * 128, :])
        ss2.append(t2)
    outv = tn["out"].ap().rearrange("(tb p) d -> p tb d", p=128)
    for tb in range(TB):
        tbb = slice(tb * 128, (tb + 1) * 128)
        for dh in range(2):
            dsl = slice(dh * 512, (dh + 1) * 512)
            py = psum.tile([128, 512], F32, tag="ps")
            for ic in range(IC):
                nc.tensor.matmul(
                    py[:], hsh[:, ic, tbb], ss2[ic][:, dsl],
                    start=(ic == 0), stop=(ic == IC - 1),
                )
            ysh = iop.tile([128, 512], F32, tag="ysh")
            nc.scalar.copy(ysh[:], py[:])
            nc.sync.dma_start(outv[:, tb, dsl], ysh[:])

    # ---- routed experts (sparse, bf16) ----
    # Tile does not thread SWDGE DMA-completion (sem +=16) into consumer
    # waits; add explicit waits: PE before reading gathered x, scatter
    # chain (RMW on out must serialize), and yf slot reuse (the scatter
    # reads yf asynchronously after its trigger).
    s_sems = []
    nfregs = []
    for e in range(E):
        nfr = nc.engines[mybir.EngineType.Pool].alloc_register(f"nfr{e}")
        nc.gpsimd.reg_load(nfr, nfound[:, 2 * e:2 * e + 1])
        nfregs.append(nfr)
        xg = xgp.tile([128, DC, CAP], BF16, tag="xg")
        g_sem = nc.alloc_semaphore(f"gat_sem{e}")
        nc.gpsimd.dma_gather(
            xg[:],
            tn["x_bf"].ap(),
            idxsc[:, e, :],
            num_idxs=CAP,
            num_idxs_reg=CAP,
            elem_size=D,
            transpose=True,
            prepare_only=True,
            sem=g_sem,
        )
        nc.gpsimd.trigger_dma(count=None)
        nc.tensor.wait_ge(g_sem, 16)
        s1 = []
        s3 = []
        for dc in range(DC):
            t1 = wpool.tile([128, INTER], BF16, tag="wslab")
            nc.sync.dma_start(t1[:], tn["w1"].ap()[e, dc * 128:(dc + 1) * 128, :])
            s1.append(t1)
            t3 = wpool.tile([128, INTER], BF16, tag="wslab")
            nc.sync.dma_start(t3[:], tn["w3"].ap()[e, dc * 128:(dc + 1) * 128, :])
            s3.append(t3)
        hs = hsp.tile([128, IC, CAP], BF16, tag="hs")
        for ic in range(IC):
            icb = slice(ic * 128, (ic + 1) * 128)
            p1 = psum.tile([128, 512], F32, tag="ps")
            p3 = psum.tile([128, 512], F32, tag="ps")
            for dc in range(DC):
                st, sp = dc == 0, dc == DC - 1
                nc.tensor.matmul(p1[:, :CAP], s1[dc][:, icb], xg[:, dc, :], start=st, stop=sp)
                nc.tensor.matmul(p3[:, :CAP], s3[dc][:, icb], xg[:, dc, :], start=st, stop=sp)
            sg = tmp.tile([128, 512], F32, tag="sg")
            if USE_SILU:
                nc.scalar.activation(sg[:, :CAP], p1[:, :CAP], AF.Silu)
            else:
                sgm = tmp.tile([128, 512], F32, tag="sgm")
                nc.scalar.activation(sgm[:, :CAP], p1[:, :CAP], AF.Sigmoid)
                nc.vector.tensor_tensor(sg[:, :CAP], sgm[:, :CAP], p1[:, :CAP], op=ALU.mult)
            nc.vector.tensor_tensor(hs[:, ic, :], sg[:, :CAP], p3[:, :CAP], op=ALU.mult)
        s2 = []
        for ic in range(IC):
            t2 = wpool.tile([128, D], BF16, tag="wslab")
            nc.sync.dma_start(t2[:], tn["w2"].ap()[e, ic * 128:(ic + 1) * 128, :])
            s2.append(t2)
        if DEBUG_DUMPS and e == 0:
            xg_dbg = nc.dram_tensor("xg_dbg", [128, DC, CAP], BF16, kind=DEBUG_KIND)
            nc.sync.dma_start(xg_dbg.ap(), xg[:])
            hs_dbg = nc.dram_tensor("hs_dbg", [128, IC, CAP], BF16, kind=DEBUG_KIND)
            nc.sync.dma_start(hs_dbg.ap(), hs[:])
        yf = yfp.tile([128, PT, D], F32, tag="yf")
        if e >= 2:
            nc.vector.wait_ge(s_sems[e - 2], 16)
        for pt in range(PT):
            pb = slice(pt * 128, (pt + 1) * 128)
            for dh in range(2):
                dsl = slice(dh * 512, (dh + 1) * 512)
                py = psum.tile([128, 512], F32, tag="ps")
                for ic in range(IC):
                    nc.tensor.matmul(
                        py[:], hs[:, ic, pb], s2[ic][:, dsl],
                        start=(ic == 0), stop=(ic == IC - 1),
                    )
                nc.vector.tensor_scalar_mul(
                    yf[:, pt, dsl], py[:], cadj[:, e, pt:pt + 1]
                )
        if DEBUG_DUMPS and e == 0:
            yf_dbg = nc.dram_tensor("yf_dbg", [128, PT, D], F32, kind=DEBUG_KIND)
            nc.sync.dma_start(yf_dbg.ap(), yf[:])
        s_sem = nc.alloc_semaphore(f"sct_sem{e}")
        s_sems.append(s_sem)
        if e > 0:
            nc.gpsimd.wait_ge(s_sems[e - 1], 16)
        nc.gpsimd.dma_scatter_add(
            tn["out"].ap(),
            yf[:],
            idxsc[:, e, :],
            num_idxs=CAP,
            num_idxs_reg=CAP,
            elem_size=D,
            prepare_only=True,
            sem=s_sem,
        )
        nc.gpsimd.trigger_dma(count=None)
    nc.gpsimd.wait_ge(s_sems[E - 1], 16)


def declare_sparse(nc):
    tn = {
        "x_bf": nc.dram_tensor("x_bf", [T, D], BF16, kind="ExternalInput"),
        "xT_bf": nc.dram_tensor("xT_bf", [D, T], BF16, kind="ExternalInput"),
        "xT_f32": nc.dram_tensor("xT_f32", [D, T], F32, kind="ExternalInput"),
        "gate_w": nc.dram_tensor("gate_w", [E, D], F32, kind="ExternalInput"),
        "tokv": nc.dram_tensor("tokv", [T], F32, kind="ExternalInput"),
        "iotaw": nc.dram_tensor("iotaw", [16, NW], F32, kind="ExternalInput"),
        "w1": nc.dram_tensor("w1", [E, D, INTER], BF16, kind="ExternalInput"),
        "w2": nc.dram_tensor("w2", [E, INTER, D], BF16, kind="ExternalInput"),
        "w3": nc.dram_tensor("w3", [E, D, INTER], BF16, kind="ExternalInput"),
        "sw1": nc.dram_tensor("sw1", [D, INTER], BF16, kind="ExternalInput"),
        "sw2": nc.dram_tensor("sw2", [INTER, D], BF16, kind="ExternalInput"),
        "sw3": nc.dram_tensor("sw3", [D, INTER], BF16, kind="ExternalInput"),
        "out": nc.dram_tensor("out", [T, D], F32, kind="ExternalOutput"),
    }
    return tn


def build_nc_sparse(num_devices=N_CORES):
    from contextlib import ExitStack

    nc = bacc.Bacc(
        "TRN2", target_bir_lowering=False, debug=False, num_devices=num_devices
    )
    tn = declare_sparse(nc)
    with tile.TileContext(nc) as tc:
        with ExitStack() as es:
            nc._emit_ctx = es
            emit_sparse(nc, tc, tn)
    nc.compile()
    return nc


def make_in_maps_sparse(inputs):
    x = np.asarray(inputs["x"], dtype=np.float32).reshape(-1, D)
    shared = {
        "gate_w": np.ascontiguousarray(np.asarray(inputs["gate_w"], np.float32)),
        "tokv": np.arange(1, T + 1, dtype=np.float32),
        "iotaw": np.ascontiguousarray(
            (16 * np.arange(NW)[None, :] + np.arange(16)[:, None]).astype(np.float32)
        ),
        "w1": _bf(inputs["w1"]),
        "w2": _bf(inputs["w2"]),
        "w3": _bf(inputs["w3"]),
        "sw1": _bf(inputs["sw1"]),
        "sw2": _bf(inputs["sw2"]),
        "sw3": _bf(inputs["sw3"]),
    }
    in_maps = []
    for c in range(N_CORES):
        xs = np.ascontiguousarray(x[c * T:(c + 1) * T])
        m = dict(shared)
        m["x_bf"] = _bf(xs)
        m["xT_bf"] = _bf(np.ascontiguousarray(xs.T))
        m["xT_f32"] = np.ascontiguousarray(xs.T)
        in_maps.append(m)
    return in_maps


def _bf(a):
    import ml_dtypes

    return np.ascontiguousarray(np.asarray(a, np.float32).astype(ml_dtypes.bfloat16))


def routed_counts(inputs):
    """Host-side capacity check mirroring the device's is_ge top-2 rule."""
    x = np.asarray(inputs["x"], np.float32).reshape(-1, D)
    gw = np.asarray(inputs["gate_w"], np.float32)
    logits = x @ gw.T
    m = logits.max(-1, keepdims=True)
    p = np.exp(logits - m)
    p /= p.sum(-1, keepdims=True)
    second = np.sort(p, axis=-1)[:, -2:-1]
    sel = p >= second
    counts = sel.reshape(N_CORES, T, E).sum(1)
    return counts


# ----------------------------------------------------------------- dense path
# (original kernel, kept as fallback for non-zero biases / capacity overflow)


def r32(ap):
    return ap.bitcast(F32R)


def emit_dense(nc, tc, tensors, zero_biases):
    NT = T // 512
    DT = D // 512
    x_d = tensors["x"]
    gate_d = tensors["gate_w"]
    out_d = tensors["out"]

    xin = x_d.ap().rearrange("(tb p) d -> p tb d", p=128)
    outv = out_d.ap().rearrange("(tb p) d -> p tb d", p=128)

    ctx = tc.nc._emit_ctx
    singles = ctx.enter_context(tc.tile_pool(name="singles", bufs=1))
    psum = ctx.enter_context(tc.tile_pool(name="psum", bufs=8, space="PSUM"))
    tmp = ctx.enter_context(tc.tile_pool(name="tmp", bufs=2))
    big = ctx.enter_context(tc.tile_pool(name="big", bufs=1))
    wpool = ctx.enter_context(tc.tile_pool(name="wpool", bufs=24))
    hpool = ctx.enter_context(tc.tile_pool(name="hpool", bufs=1))
    iop = ctx.enter_context(tc.tile_pool(name="iop", bufs=6))

    ident = singles.tile([128, 128], F32)
    masks.make_identity(nc, ident[:])
    onesf = singles.tile([1, 128], F32)
    nc.vector.memset(onesf[:], 1.0)
    ones1 = singles.tile([1, 128], F32R)
    nc.vector.tensor_copy(ones1[:], onesf[:])

    gwT = singles.tile([128, DC, E], F32)
    for dc in range(DC):
        nc.sync.dma_start(
            gwT[:, dc, :],
            gate_d.ap()[:, dc * 128:(dc + 1) * 128].rearrange("e p -> p e"),
        )

    ZB = zero_biases
    b1s = b3s = sb1s = sb3s = b2r = sb2r = None
    if not ZB:
        b1s = singles.tile([128, E, IC], F32)
        b3s = singles.tile([128, E, IC], F32)
        for e in range(E):
            nc.sync.dma_start(
                b1s[:, e, :],
                tensors["b1"].ap()[e].rearrange("(ic p) -> p ic", p=128),
            )
            nc.sync.dma_start(
                b3s[:, e, :],
                tensors["b3"].ap()[e].rearrange("(ic p) -> p ic", p=128),
            )
        sb1s = singles.tile([128, IC], F32)
        nc.sync.dma_start(
            sb1s[:], tensors["sb1"].ap().rearrange("(ic p) -> p ic", p=128)
        )
        sb3s = singles.tile([128, IC], F32)
        nc.sync.dma_start(
            sb3s[:], tensors["sb3"].ap().rearrange("(ic p) -> p ic", p=128)
        )
        b2r = singles.tile([E, D], F32R)
        nc.sync.dma_start(b2r[:], tensors["b2"].ap().bitcast(F32R))
        sb2r = singles.tile([1, D], F32R)
        nc.sync.dma_start(
            sb2r[:],
            tensors["sb2"].ap().rearrange("(o d) -> o d", o=1).bitcast(F32R),
        )

    xt = big.tile([128, DC, T], F32R)
    comb = big.tile([128, TB, E], F32)
    comb_t = None if ZB else big.tile([8, T], F32R)

    xpool_cm = tc.tile_pool(name="xnat", bufs=2)
    xpool = xpool_cm.__enter__()
    for tb in range(TB):
        xnat = xpool.tile([128, D], F32, tag="xnat")
        nc.sync.dma_start(xnat[:], xin[:, tb, :])
        xstage = xpool.tile([128, DC, 128], F32, tag="xstage")
        for dc in range(DC):
            pt = psum.tile([128, 512], F32, tag="ps")
            nc.tensor.transpose(
                pt[:, :128], xnat[:, dc * 128:(dc + 1) * 128], ident[:]
            )
            nc.vector.tensor_copy(xstage[:, dc, :], pt[:, :128])
            nc.vector.tensor_copy(xt[:, dc, tb * 128:(tb + 1) * 128], xstage[:, dc, :])
        ps = psum.tile([128, 512], F32, tag="ps")
        for dc in range(DC):
            nc.tensor.matmul(
                ps[:, :E],
                xstage[:, dc, :],
                gwT[:, dc, :],
                start=(dc == 0),
                stop=(dc == DC - 1),
            )
        nmx = tmp.tile([128, 1], F32, tag="nmx")
        nc.vector.tensor_reduce(nmx[:], ps[:, :E], axis=AX.X, op=ALU.max, negate=True)
        ex = tmp.tile([128, E], F32, tag="ex")
        nc.scalar.activation(ex[:], ps[:, :E], AF.Exp, bias=nmx[:])
        ssum = tmp.tile([128, 1], F32, tag="ssum")
        nc.vector.tensor_reduce(ssum[:], ex[:], axis=AX.X, op=ALU.add)
        rs = tmp.tile([128, 1], F32, tag="rs")
        nc.vector.reciprocal(rs[:], ssum[:])
        probs = tmp.tile([128, E], F32, tag="probs")
        nc.vector.tensor_scalar_mul(probs[:], ex[:], rs[:])
        m8 = tmp.tile([128, 8], F32, tag="m8")
        nc.vector.max(m8[:], probs[:])
        msk = tmp.tile([128, E], F32, tag="msk")
        nc.vector.tensor_scalar(msk[:], probs[:], m8[:, 1:2], None, op0=ALU.is_ge)
        nc.vector.tensor_mul(comb[:, tb, :], probs[:], msk[:])
        if not ZB:
            ptc = psum.tile([128, 512], F32, tag="ps")
            nc.tensor.transpose(ptc[:8, :128], comb[:, tb, :], ident[:])
            nc.vector.tensor_copy(
                comb_t[:, tb * 128:(tb + 1) * 128], ptc[:8, :128]
            )

    xpool_cm.__exit__(None, None, None)

    for j in range(-1, E):
        shared = j < 0
        hbuf = hpool.tile([128, IC, T], F32R, tag="hbuf")
        if shared:
            w1d, w3d, w2d = tensors["sw1"].ap(), tensors["sw3"].ap(), tensors["sw2"].ap()
        else:
            w1d, w3d, w2d = (
                tensors["w1"].ap()[j],
                tensors["w3"].ap()[j],
                tensors["w2"].ap()[j],
            )

        s1 = []
        s3 = []
        for dc in range(DC):
            t1 = wpool.tile([128, INTER], F32R, tag="wslab")
            nc.sync.dma_start(t1[:], w1d[dc * 128:(dc + 1) * 128, :].bitcast(F32R))
            s1.append(t1)
            t3 = wpool.tile([128, INTER], F32R, tag="wslab")
            nc.sync.dma_start(t3[:], w3d[dc * 128:(dc + 1) * 128, :].bitcast(F32R))
            s3.append(t3)

        for nt in range(NT):
            tsl = slice(nt * 512, (nt + 1) * 512)
            for icp in range(IC // 2):
                phs = []
                for k in range(2):
                    ic = icp * 2 + k
                    icb = slice(ic * 128, (ic + 1) * 128)
                    p1 = psum.tile([128, 512], F32, tag="ps")
                    p3 = psum.tile([128, 512], F32, tag="ps")
                    for dc in range(DC):
                        st, sp = dc == 0, dc == DC - 1
                        nc.tensor.matmul(
                            p1[:], s1[dc][:, icb], xt[:, dc, tsl],
                            start=st, stop=sp,
                        )
                        nc.tensor.matmul(
                            p3[:], s3[dc][:, icb], xt[:, dc, tsl],
                            start=st, stop=sp,
                        )
                    phs.append((ic, p1, p3))
                for ic, p1, p3 in phs:
                    hs = tmp.tile([128, 512], F32, tag="hs")
                    if ZB:
                        nc.scalar.activation(hs[:], p1[:], AF.Silu)
                        nc.vector.tensor_mul(hbuf[:, ic, tsl], hs[:], p3[:])
                        continue
                    b1c = sb1s[:, ic:ic + 1] if shared else b1s[:, j, ic:ic + 1]
                    b3c = sb3s[:, ic:ic + 1] if shared else b3s[:, j, ic:ic + 1]
                    t3v = tmp.tile([128, 512], F32, tag="t3v")
                    nc.vector.tensor_scalar_add(t3v[:], p3[:], b3c)
                    nc.scalar.activation(hs[:], p1[:], AF.Silu, bias=b1c)
                    nc.vector.tensor_mul(hbuf[:, ic, tsl], hs[:], t3v[:])

        s2 = []
        for ic in range(IC):
            t2 = wpool.tile([128, D], F32R, tag="wslab")
            nc.sync.dma_start(t2[:], w2d[ic * 128:(ic + 1) * 128, :].bitcast(F32R))
            s2.append(t2)
        b2row = None if ZB else (sb2r[0:1, :] if shared else b2r[j:j + 1, :])
        for tb in range(TB):
            tbb = slice(tb * 128, (tb + 1) * 128)
            for dt in range(DT):
                dsl = slice(dt * 512, (dt + 1) * 512)
                py = psum.tile([128, 512], F32, tag="ps")
                for ic in range(IC):
                    nc.tensor.matmul(
                        py[:], hbuf[:, ic, tbb], s2[ic][:, dsl],
                        start=(ic == 0),
                        stop=(ic == IC - 1) and (ZB or not shared),
                    )
                if not ZB and shared:
                    nc.tensor.matmul(
                        py[:], ones1[:], b2row[:, dsl],
                        start=False, stop=False,
                    )
                    nc.tensor.matmul(
                        py[:], comb_t[:, tbb], b2r[:, dsl],
                        start=False, stop=True,
                    )
                st = iop.tile([128, 512], F32, tag="st")
                if shared:
                    nc.scalar.copy(st[:], py[:])
                else:
                    nc.vector.tensor_scalar_mul(st[:], py[:], comb[:, tb, j:j + 1])
                    rd = iop.tile([128, 512], F32, tag="rd")
                    nc.sync.dma_start(rd[:], outv[:, tb, dsl])
                    nc.vector.tensor_tensor(st[:], st[:], rd[:], op=ALU.add)
                nc.sync.dma_start(outv[:, tb, dsl], st[:])


def declare_dense(nc):
    tensors = {
        "x": nc.dram_tensor("x", [T, D], F32, kind="ExternalInput"),
        "gate_w": nc.dram_tensor("gate_w", [E, D], F32, kind="ExternalInput"),
        "w1": nc.dram_tensor("w1", [E, D, INTER], F32, kind="ExternalInput"),
        "b1": nc.dram_tensor("b1", [E, INTER], F32, kind="ExternalInput"),
        "w2": nc.dram_tensor("w2", [E, INTER, D], F32, kind="ExternalInput"),
        "b2": nc.dram_tensor("b2", [E, D], F32, kind="ExternalInput"),
        "w3": nc.dram_tensor("w3", [E, D, INTER], F32, kind="ExternalInput"),
        "b3": nc.dram_tensor("b3", [E, INTER], F32, kind="ExternalInput"),
        "sw1": nc.dram_tensor("sw1", [D, INTER], F32, kind="ExternalInput"),
        "sb1": nc.dram_tensor("sb1", [INTER], F32, kind="ExternalInput"),
        "sw2": nc.dram_tensor("sw2", [INTER, D], F32, kind="ExternalInput"),
        "sb2": nc.dram_tensor("sb2", [D], F32, kind="ExternalInput"),
        "sw3": nc.dram_tensor("sw3", [D, INTER], F32, kind="ExternalInput"),
        "sb3": nc.dram_tensor("sb3", [INTER], F32, kind="ExternalInput"),
        "out": nc.dram_tensor("out", [T, D], F32, kind="ExternalOutput"),
    }
    return tensors


def build_nc_dense(zero_biases, num_devices=N_CORES):
    from contextlib import ExitStack

    nc = bacc.Bacc(
        "TRN2", target_bir_lowering=False, debug=False, num_devices=num_devices
    )
    tensors = declare_dense(nc)
    with tile.TileContext(nc) as tc:
        with ExitStack() as es:
            nc._emit_ctx = es
            emit_dense(nc, tc, tensors, zero_biases)
    nc.compile()
    return nc


def make_in_maps_dense(inputs):
    x = np.ascontiguousarray(
        np.asarray(inputs["x"], dtype=np.float32).reshape(-1, D)
    )
    shared_names = [
        "gate_w", "w1", "b1", "w2", "b2", "w3", "b3",
        "sw1", "sb1", "sw2", "sb2", "sw3", "sb3",
    ]
    shared = {
        k: np.ascontiguousarray(np.asarray(inputs[k], dtype=np.float32))
        for k in shared_names
    }
    in_maps = []
    for c in range(N_CORES):
        m = dict(shared)
        m["x"] = np.ascontiguousarray(x[c * T:(c + 1) * T])
        in_maps.append(m)
    return in_maps


# --------------------------------------------------------------------- driver


def kernel(**inputs) -> np.ndarray:
    zero_biases = all(
        not np.any(np.asarray(inputs[k]))
        for k in ("b1", "b2", "b3", "sb1", "sb2", "sb3")
    )
    use_sparse = zero_biases and routed_counts(inputs).max() <= CAP
    if use_sparse:
        nc = build_nc_sparse()
        in_maps = make_in_maps_sparse(inputs)
    else:
        nc = build_nc_dense(zero_biases)
        in_maps = make_in_maps_dense(inputs)
    res = run_bass_kernel_spmd(nc, in_maps, core_ids=list(range(N_CORES)))
    out = np.concatenate([res.results[c]["out"] for c in range(N_CORES)], axis=0)
    return out.reshape(np.asarray(inputs["x"]).shape)
